# revision 10
# baseline (speedup 1.0000x reference)
"""Trainium2 Bass kernel: Whisper-style self-attention (B=4, S=1500, D=1280, H=20).

Sharding: core c = 2*b + g handles batch b (of 4) and head-group g (of 2,
10 heads each).  Every matmul is exactly 1/8 of the total work:
  - Q/K/V projections column-sharded over the head group,
  - attention sharded by (batch, head),
  - output projection row-sharded; the two head-group partials of each batch
    are summed on the host (plus bias terms, which fold into host math).

Device dataflow (per core), all fp16 operands (PSUM f32):
  xT [1280,1500] -> qT,kT [640,1500] fp16 (qT scaled 1/8 + bq),
  v [1500,10,65] (64 v cols + ones col per head -> softmax Z).
  Per (head h, sq chunk c): scoresT = kT.T@qT per (128-row k tile), Exp
  batched over psum bank pairs on ACT -> expT fp16.  Then per 128-col sq
  subtile: ctx[sq,65] accumulated in PSUM with ex as the STATIONARY operand
  (12 matmuls of only 65 moving cols each - 2x fewer PE cycles than
  streaming expT), DVE reciprocal of the Z column + per-partition
  tensor_scalar multiply -> ctx_sb fp16 [sq,128] (head pair), then a DMA
  transpose (xbar) writes ctxT [128,sq] directly - no PE transpose, no
  PSUM->SBUF copy.  O-proj fp16 (ctxT stationary, wo moving).

Scheduling: sequencers are in-order and sem waits hold the SEQ, so the
emission order IS the schedule.  Attention units (software-pipelined one
unit deep) are ACT-bound: per 512-col chunk ACT needs ~6.2us of exp while
its scores+attnV PE work is only ~3.9us, and the ps2 WAR (bufs=2) stalls
PE inside the scores loop ~0.6us per psum pair.  So ALL other PE work
(projections, O-proj) is chopped into one-PSUM-group "micro" pieces and a
credit scheduler pumps ~one micro per scores pair, subject to
read-after-write deadlines (dl) and transpose-gating (nb).  ctx transposes
own the SP DMA queue; out stores ride Pool/SWDGE; weight loads are split
per 128-col slice so the first matmuls start ~2us after launch.
"""
import sys
sys.path.insert(0, "/opt/trn_rl_repo")

from collections import deque
from contextlib import ExitStack
import numpy as np

import concourse.bass as bass
import concourse.tile as tile
from concourse import bacc, mybir
from concourse.bass_utils import run_bass_kernel_spmd

dt = mybir.dt
AF = mybir.ActivationFunctionType
ALU = mybir.AluOpType

N_CORES = 8
B, S, D = 4, 1500, 1280
H, DH = 20, 64
G = 2
DG = D // G           # 640
HPG = H // G          # 10
KD = D // 128         # 10
MD = DG // 128        # 5
CW = (512, 512, 476)  # sq/proj chunk widths (PSUM-bank bound)
CO = (0, 512, 1024)   # chunk offsets
NS = 3
KS = (S + 127) // 128  # 12 (11*128 + 92)
ON = (512, 512, 256)
OO = (0, 512, 1024)
SP = S + 4            # ctxT padded to 1504 so the last 96-wide sq subtile
                      # (92 real cols) can be DMA-transposed whole

_CACHE = {}


def _sk(i):
    return min(128, S - i * 128)


def _subtiles(c):
    """(local_off, width) 128-col subtiles of chunk c; last one padded to 96."""
    w = CW[c]
    out = []
    off = 0
    while off < w:
        sw = min(128, w - off)
        if sw % 16:
            sw = 96  # pad 92 -> 96 for the xbar transpose (junk cols unread)
        out.append((off, sw))
        off += 128
    return out


def build():
    nc = bacc.Bacc("TRN2", target_bir_lowering=False, debug=False,
                   num_devices=N_CORES)
    xt_d = nc.dram_tensor("xt", [D, S], dt.float16, kind="ExternalInput").ap()
    wq_d = nc.dram_tensor("wq", [D, DG], dt.float16, kind="ExternalInput").ap()
    wk_d = nc.dram_tensor("wk", [D, DG], dt.float16, kind="ExternalInput").ap()
    wv_d = nc.dram_tensor("wv", [D, DG], dt.float16, kind="ExternalInput").ap()
    wo_d = nc.dram_tensor("wo", [DG, D], dt.float16, kind="ExternalInput").ap()
    bq_d = nc.dram_tensor("bq", [128, MD], dt.float32, kind="ExternalInput").ap()
    out_d = nc.dram_tensor("out", [S, D], dt.float16, kind="ExternalOutput").ap()

    xt_r = xt_d.rearrange("(k p) s -> p k s", p=128)
    wq_r = wq_d.rearrange("(k p) n -> p k n", p=128)
    wk_r = wk_d.rearrange("(k p) n -> p k n", p=128)
    wv_r = wv_d.rearrange("(k p) n -> p k n", p=128)
    wo_r = wo_d.rearrange("(k p) n -> p k n", p=128)

    with tile.TileContext(nc) as tc, ExitStack() as octx:
        persist = octx.enter_context(tc.tile_pool(name="persist", bufs=1))
        epool = octx.enter_context(tc.tile_pool(name="expT", bufs=3))
        zpool = octx.enter_context(tc.tile_pool(name="z", bufs=3))
        cpool = octx.enter_context(tc.tile_pool(name="ctxsb", bufs=12))
        opool = octx.enter_context(tc.tile_pool(name="ob", bufs=3))
        ps2 = octx.enter_context(tc.tile_pool(name="ps2", bufs=2, space="PSUM"))
        ps1 = octx.enter_context(tc.tile_pool(name="ps1", bufs=2, space="PSUM"))
        pat = octx.enter_context(tc.tile_pool(name="pat", bufs=2, space="PSUM"))

        qT = persist.tile([128, MD, S], dt.float16, tag="qT")
        kT = persist.tile([128, MD, S], dt.float16, tag="kT")
        v = persist.tile([128, KS, HPG, DH + 1], dt.float16, tag="v")
        ctxT = persist.tile([128, MD, SP], dt.float16, tag="ctxT")
        bq_s = persist.tile([128, MD], dt.float32, tag="bq")
        xt_s = persist.tile([128, KD, S], dt.float16, tag="xt")
        wqs = persist.tile([128, KD, DG], dt.float16, tag="wqs")
        wks = persist.tile([128, KD, DG], dt.float16, tag="wks")
        wvs = persist.tile([128, KD, DG], dt.float16, tag="wvs")
        wo_s = persist.tile([128, MD, D], dt.float16, tag="wo")

        # --- input DMAs, ordered for earliest first matmul (the cost model
        # serializes transfers on one DMA_ENGINES slot, so order matters)
        nc.sync.dma_start(out=bq_s[:], in_=bq_d[:])
        nc.sync.dma_start(out=wks[:, :, 0:128], in_=wk_r[:, :, 0:128])
        for k2 in range(0, KD, 2):  # chunk 0 split so QK matmuls start early
            nc.sync.dma_start(out=xt_s[:, k2:k2 + 2, 0:CW[0]],
                              in_=xt_r[:, k2:k2 + 2, 0:CW[0]])
        nc.sync.dma_start(out=wqs[:, :, 0:128], in_=wq_r[:, :, 0:128])
        nc.sync.dma_start(out=wvs[:, :, 0:128], in_=wv_r[:, :, 0:128])
        for n in (1, 2):
            nsl = slice(CO[n], CO[n] + CW[n])
            nc.sync.dma_start(out=xt_s[:, :, nsl], in_=xt_r[:, :, nsl])
        for m in range(1, MD):
            sl = slice(m * 128, (m + 1) * 128)
            nc.sync.dma_start(out=wks[:, :, sl], in_=wk_r[:, :, sl])
            nc.sync.dma_start(out=wqs[:, :, sl], in_=wq_r[:, :, sl])
            nc.sync.dma_start(out=wvs[:, :, sl], in_=wv_r[:, :, sl])

        ones1 = persist.tile([128, 1], dt.float16, tag="ones1")
        nc.vector.memset(ones1[:], 1.0)
        nc.vector.tensor_copy(v[:, :, :, DH:DH + 1],
                              ones1[:].to_broadcast([128, KS, HPG, 1]))

        # ---- micro building blocks (one ps1 PSUM group each) -----------
        def qk_micro(m, which, n):
            """One sq chunk of the q or k projection for d-tile m (~2.1us)."""
            ws = wqs if which == "q" else wks
            cw, co = CW[n], CO[n]
            ps = ps1.tile([128, 1, 512], dt.float32, tag="ps1", name="ps1")
            for kk in range(KD):
                nc.tensor.matmul(
                    ps[:, 0, 0:cw],
                    lhsT=ws[:, kk, m * 128:(m + 1) * 128],
                    rhs=xt_s[:, kk, co:co + cw],
                    start=(kk == 0), stop=(kk == KD - 1))
            if which == "q":
                nc.vector.tensor_scalar(
                    qT[:, m, co:co + cw], ps[:, 0, 0:cw], 0.125,
                    bq_s[:, m:m + 1], op0=ALU.mult, op1=ALU.add)
            else:
                nc.vector.tensor_copy(kT[:, m, co:co + cw], ps[:, 0, 0:cw])

        def v_micro(hp, ms):
            """v columns for head pair hp, one 128-row s tile (~0.6us)."""
            sp = _sk(ms)
            ps = ps1.tile([128, 1, 512], dt.float32, tag="ps1", name="ps1")
            for kk in range(KD):
                nc.tensor.matmul(
                    ps[0:sp, 0, 0:128],
                    lhsT=xt_s[:, kk, ms * 128:ms * 128 + sp],
                    rhs=wvs[:, kk, hp * 128:(hp + 1) * 128],
                    start=(kk == 0), stop=(kk == KD - 1))
            nc.vector.tensor_copy(
                v[0:sp, ms, 2 * hp:2 * hp + 2, 0:DH],
                ps[0:sp, 0, 0:128].rearrange("p (h e) -> p h e", h=2))

        def wo_micro():
            nc.sync.dma_start(out=wo_s[:], in_=wo_r[:])

        def op_micro(ms, j):
            """One 512-col group of the O-projection for sq tile ms."""
            sp = _sk(ms)
            nw, noff = ON[j], OO[j]
            ps = ps1.tile([128, 1, 512], dt.float32, tag="ps1", name="ps1")
            for kk in range(MD):
                nc.tensor.matmul(
                    ps[0:sp, 0, 0:nw],
                    lhsT=ctxT[:, kk, ms * 128:ms * 128 + sp],
                    rhs=wo_s[:, kk, noff:noff + nw],
                    start=(kk == 0), stop=(kk == MD - 1))
            ob = opool.tile([128, 512], dt.float16, tag="ob", name="ob")
            nc.vector.tensor_copy(ob[0:sp, 0:nw], ps[0:sp, 0, 0:nw])
            # out-stores ride Pool/SWDGE: SP.SEQ is busy with ctx transposes
            # whose sem waits hold it.
            nc.gpsimd.dma_start(
                out=out_d[ms * 128:ms * 128 + sp, noff:noff + nw],
                in_=ob[0:sp, 0:nw])

        # ---- attention unit pieces -------------------------------------
        def emit_scores(h, c, pump):
            base = 64 * (h % 2)
            td = h // 2
            cw, co = CW[c], CO[c]
            csl = slice(co, co + cw)
            ex = epool.tile([128, KS, 512], dt.float16, tag="expT", name="ex")
            for kk2 in range(0, KS, 2):
                ps = ps2.tile([128, 2, 512], dt.float32, tag="ps2", name="ps2")
                for j in range(2):
                    kk = kk2 + j
                    sp = _sk(kk)
                    nc.tensor.matmul(
                        ps[0:sp, j, 0:cw],
                        lhsT=kT[base:base + 64, td, kk * 128:kk * 128 + sp],
                        rhs=qT[base:base + 64, td, csl],
                        start=True, stop=True)
                nc.scalar.activation(ex[:, kk2:kk2 + 2, 0:cw], ps[:, :, 0:cw],
                                     AF.Exp)
                pump()
            return ex

        csb_live = {}

        def emit_tail(h, c, ex):
            """attnV (ex stationary) + 1/Z scale into the pair's ctx_sb."""
            td, hb = h // 2, 64 * (h % 2)
            if (td, c) not in csb_live:
                csb_live[(td, c)] = {
                    off: cpool.tile([128, 128], dt.float16, tag="ctxsb",
                                    name="ctxsb")
                    for off, _ in _subtiles(c)}
            csb = csb_live[(td, c)]
            for off, sw in _subtiles(c):
                pc = pat.tile([128, DH + 1], dt.float32, tag="pat", name="pat")
                for kk in range(KS):
                    sp = _sk(kk)
                    nc.tensor.matmul(
                        pc[0:sw, :],
                        lhsT=ex[0:sp, kk, off:off + sw],
                        rhs=v[0:sp, kk, h, :],
                        start=(kk == 0), stop=(kk == KS - 1))
                rz = zpool.tile([128, 1], dt.float32, tag="rz", name="rz")
                nc.vector.reciprocal(rz[0:sw, :], pc[0:sw, DH:DH + 1])
                nc.vector.tensor_scalar(
                    csb[off][0:sw, hb:hb + 64], pc[0:sw, 0:DH], rz[0:sw, :],
                    None, op0=ALU.mult)
            if h % 2 == 1:  # pair complete: transpose ctx into ctxT
                for off, sw in _subtiles(c):
                    nc.sync.dma_start(
                        out=ctxT[:, td, CO[c] + off:CO[c] + off + sw],
                        in_=csb[off][0:sw, :], transpose=True)
                del csb_live[(td, c)]

        # ---- schedule --------------------------------------------------
        # c-major pair order: all c0 pairs first => O-proj for sq<512 can
        # start as filler at iteration 11, sq<1024 at 21.
        pairs = [(td, c) for c in (0, 1, 2) for td in range(5)]
        units = [(2 * td + o, c) for td, c in pairs for o in (0, 1)]

        # micro list: (cost_rows, dl, nb, fn); consumed strictly in order.
        # dl: must be emitted before scores of that iteration (RAW via
        # emission order).  nb: not before that iteration (transpose gating).
        M = []
        for ms in range(KS):
            M.append((1280, 1, 0, lambda ms=ms: v_micro(0, ms)))
        for m in range(1, MD):
            for n in range(NS):
                M.append((10 * CW[n], 2 * m, 0,
                          lambda m=m, n=n: qk_micro(m, "k", n)))
            M.append((10 * CW[0], 2 * m, 0, lambda m=m: qk_micro(m, "q", 0)))
            for ms in range(KS):
                M.append((1280, 2 * m + 1, 0,
                          lambda m=m, ms=ms: v_micro(m, ms)))
        M.append((0, 9, 0, wo_micro))
        for m in range(0, MD):
            M.append((10 * CW[1], 10 + 2 * m, 0,
                      lambda m=m: qk_micro(m, "q", 1)))
            if m == 1:
                for ms in (0, 1):
                    for j in range(NS):
                        M.append((5 * ON[j], 29, 11,
                                  lambda ms=ms, j=j: op_micro(ms, j)))
            if m == 2:
                for ms in (2, 3):
                    for j in range(NS):
                        M.append((5 * ON[j], 29, 11,
                                  lambda ms=ms, j=j: op_micro(ms, j)))
        for m in range(0, MD):
            M.append((10 * CW[2], 20 + 2 * m, 0,
                      lambda m=m: qk_micro(m, "q", 2)))
            if m in (1, 2):
                for ms in (2 * m + 2, 2 * m + 3):
                    for j in range(NS):
                        M.append((5 * ON[j], 29, 21,
                                  lambda ms=ms, j=j: op_micro(ms, j)))
        mq = deque(M)
        pace = sum(c for c, _, _, _ in M) / (len(units) * 6.0)

        state = {"iter": 0, "debt": 0.0}

        def drain_deadlines():
            # pop through the LAST due micro (due ones may sit behind
            # not-yet-due ops in the strictly-ordered queue)
            it = state["iter"]
            idx = -1
            for k, m in enumerate(mq):
                if m[1] <= it:
                    idx = k
            for _ in range(idx + 1):
                _, _, nb, fn = mq.popleft()
                assert nb <= it, "nb violation forced by a deadline"
                fn()

        def pump():
            state["debt"] += pace
            while mq and state["debt"] > 0 and mq[0][2] <= state["iter"]:
                cost, _, _, fn = mq.popleft()
                fn()
                state["debt"] -= cost

        # prelude: kT d-tile 0 (scores need all of kT) + qT d-tile 0 chunk 0
        for n in range(NS):
            qk_micro(0, "k", n)
        qk_micro(0, "q", 0)

        exm = {}
        for i, u in enumerate(units):
            state["iter"] = i
            drain_deadlines()
            exm[u] = emit_scores(u[0], u[1], pump)
            if i >= 1:
                up = units[i - 1]
                emit_tail(up[0], up[1], exm.pop(up))
        up = units[-1]
        emit_tail(up[0], up[1], exm.pop(up))
        state["iter"] = len(units)
        while mq:
            _, _, _, fn = mq.popleft()
            fn()
        for ms in range(8, KS):
            for j in range(NS):
                op_micro(ms, j)

    nc.compile()
    return nc


def _get_nc():
    if "nc" not in _CACHE:
        _CACHE["nc"] = build()
    return _CACHE["nc"]


def _prep_in_maps(x, Wq, bq, Wk, Wv, Wo):
    in_maps = []
    for c in range(N_CORES):
        b, g = divmod(c, G)
        gs = slice(g * DG, (g + 1) * DG)
        in_maps.append({
            "xt": np.ascontiguousarray(x[b].T).astype(np.float16),
            "wq": np.ascontiguousarray(Wq[gs, :].T).astype(np.float16),
            "wk": np.ascontiguousarray(Wk[gs, :].T).astype(np.float16),
            "wv": np.ascontiguousarray(Wv[gs, :].T).astype(np.float16),
            "wo": np.ascontiguousarray(Wo[:, gs].T).astype(np.float16),
            "bq": np.ascontiguousarray(
                (0.125 * bq[gs]).astype(np.float32).reshape(MD, 128).T),
        })
    return in_maps


def run(x, Wq, bq, Wk, Wv, bv, Wo, bo, trace=False, **trace_kw):
    x = np.asarray(x, dtype=np.float32)
    Wq = np.asarray(Wq, dtype=np.float32)
    bq = np.asarray(bq, dtype=np.float32)
    Wk = np.asarray(Wk, dtype=np.float32)
    Wv = np.asarray(Wv, dtype=np.float32)
    bv = np.asarray(bv, dtype=np.float32)
    Wo = np.asarray(Wo, dtype=np.float32)
    bo = np.asarray(bo, dtype=np.float32)

    nc = _get_nc()
    in_maps = _prep_in_maps(x, Wq, bq, Wk, Wv, Wo)
    res = None
    for attempt in range(3):
        try:
            res = run_bass_kernel_spmd(nc, in_maps, list(range(N_CORES)),
                                       trace=trace, **trace_kw)
            break
        except Exception:
            # Sporadic NRT_EXEC_UNIT_UNRECOVERABLE on first exec; devices
            # come back after ~75s. Reset the backend and retry.
            if attempt == 2:
                raise
            import time as _time
            import jax as _jax
            _time.sleep(80)
            try:
                _jax.clear_backends()
            except Exception:
                pass
    const = (bv @ Wo.T + bo).astype(np.float32)  # [D]
    out = np.empty((B, S, D), dtype=np.float32)
    for b in range(B):
        out[b] = (res.results[2 * b]["out"].astype(np.float32)
                  + res.results[2 * b + 1]["out"].astype(np.float32) + const)
    return out, res


def kernel(**inputs):
    out, _ = run(**inputs)
    return out


# revision 14
# speedup vs baseline: 1.0579x; 1.0579x over previous
"""Trainium2 Bass kernel: Whisper-style self-attention (B=4, S=1500, D=1280, H=20).

Sharding: core c = 2*b + g handles batch b (of 4) and head-group g (of 2,
10 heads each).  Every matmul is exactly 1/8 of the total work:
  - Q/K/V projections column-sharded over the head group,
  - attention sharded by (batch, head),
  - output projection row-sharded; the two head-group partials of each batch
    are summed on the host (plus bias terms, which fold into host math).

Device dataflow (per core), all fp16 operands (PSUM f32):
  xT [1280,1500] -> qT,kT [640,1500] fp16 (qT scaled 1/8 + bq),
  v [1500,10,65] (64 v cols + ones col per head -> softmax Z).
  Per (head h, sq chunk c): scoresT = kT.T@qT per (128-row k tile), Exp
  batched over psum bank pairs on ACT -> expT fp16.  Then per 128-col sq
  subtile: ctx[sq,65] accumulated in PSUM with ex as the STATIONARY operand
  (12 matmuls of only 65 moving cols each - 2x fewer PE cycles than
  streaming expT), DVE reciprocal of the Z column + per-partition
  tensor_scalar multiply -> ctx_sb fp16 [sq,128] (head pair), then a DMA
  transpose (xbar) writes ctxT [128,sq] directly - no PE transpose, no
  PSUM->SBUF copy.  O-proj fp16 (ctxT stationary, wo moving).

Scheduling: sequencers are in-order and sem waits hold the SEQ, so the
emission order IS the schedule.  Attention units (software-pipelined one
unit deep) are ACT-bound: per 512-col chunk ACT needs ~6.2us of exp while
its scores+attnV PE work is only ~3.9us, and the ps2 WAR (bufs=2) stalls
PE inside the scores loop ~0.6us per psum pair.  So ALL other PE work
(projections, O-proj) is chopped into one-PSUM-group "micro" pieces and a
credit scheduler pumps ~one micro per scores pair, subject to
read-after-write deadlines (dl) and transpose-gating (nb).  ctx transposes
own the SP DMA queue; out stores ride Pool/SWDGE; weight loads are split
per 128-col slice so the first matmuls start ~2us after launch.
"""
import sys
sys.path.insert(0, "/opt/trn_rl_repo")

from collections import deque
from contextlib import ExitStack
import numpy as np

import concourse.bass as bass
import concourse.tile as tile
from concourse import bacc, mybir
from concourse.bass_utils import run_bass_kernel_spmd

dt = mybir.dt
AF = mybir.ActivationFunctionType
ALU = mybir.AluOpType

N_CORES = 8
B, S, D = 4, 1500, 1280
H, DH = 20, 64
G = 2
DG = D // G           # 640
HPG = H // G          # 10
KD = D // 128         # 10
MD = DG // 128        # 5
CW = (512, 512, 476)  # sq/proj chunk widths (PSUM-bank bound)
CO = (0, 512, 1024)   # chunk offsets
NS = 3
KS = (S + 127) // 128  # 12 (11*128 + 92)
ON = (512, 512, 256)
OO = (0, 512, 1024)
SP = S + 4            # ctxT padded to 1504 so the last 96-wide sq subtile
                      # (92 real cols) can be DMA-transposed whole

_CACHE = {}


def _sk(i):
    return min(128, S - i * 128)


def _subtiles(c):
    """(local_off, width) 128-col subtiles of chunk c; last one padded to 96."""
    w = CW[c]
    out = []
    off = 0
    while off < w:
        sw = min(128, w - off)
        if sw % 16:
            sw = 96  # pad 92 -> 96 for the xbar transpose (junk cols unread)
        out.append((off, sw))
        off += 128
    return out


def build():
    nc = bacc.Bacc("TRN2", target_bir_lowering=False, debug=False,
                   num_devices=N_CORES)
    xt_d = nc.dram_tensor("xt", [D, S], dt.float16, kind="ExternalInput").ap()
    wq_d = nc.dram_tensor("wq", [D, DG], dt.float16, kind="ExternalInput").ap()
    wk_d = nc.dram_tensor("wk", [D, DG], dt.float16, kind="ExternalInput").ap()
    wv_d = nc.dram_tensor("wv", [D, DG], dt.float16, kind="ExternalInput").ap()
    wo_d = nc.dram_tensor("wo", [DG, D], dt.float16, kind="ExternalInput").ap()
    bq_d = nc.dram_tensor("bq", [128, MD], dt.float32, kind="ExternalInput").ap()
    out_d = nc.dram_tensor("out", [S, D], dt.float16, kind="ExternalOutput").ap()

    xt_r = xt_d.rearrange("(k p) s -> p k s", p=128)
    wq_r = wq_d.rearrange("(k p) n -> p k n", p=128)
    wk_r = wk_d.rearrange("(k p) n -> p k n", p=128)
    wv_r = wv_d.rearrange("(k p) n -> p k n", p=128)
    wo_r = wo_d.rearrange("(k p) n -> p k n", p=128)

    with tile.TileContext(nc) as tc, ExitStack() as octx:
        persist = octx.enter_context(tc.tile_pool(name="persist", bufs=1))
        epool = octx.enter_context(tc.tile_pool(name="expT", bufs=3))
        zpool = octx.enter_context(tc.tile_pool(name="z", bufs=3))
        cpool = octx.enter_context(tc.tile_pool(name="ctxsb", bufs=12))
        opool = octx.enter_context(tc.tile_pool(name="ob", bufs=3))
        ps2 = octx.enter_context(tc.tile_pool(name="ps2", bufs=2, space="PSUM"))
        ps1 = octx.enter_context(tc.tile_pool(name="ps1", bufs=2, space="PSUM"))
        pat = octx.enter_context(tc.tile_pool(name="pat", bufs=2, space="PSUM"))

        qT = persist.tile([128, MD, S], dt.float16, tag="qT")
        kT = persist.tile([128, MD, S], dt.float16, tag="kT")
        v = persist.tile([128, KS, HPG, DH + 1], dt.float16, tag="v")
        ctxT = persist.tile([128, MD, SP], dt.float16, tag="ctxT")
        bq_s = persist.tile([128, MD], dt.float32, tag="bq")
        xt_s = persist.tile([128, KD, S], dt.float16, tag="xt")
        wqs = persist.tile([128, KD, DG], dt.float16, tag="wqs")
        wks = persist.tile([128, KD, DG], dt.float16, tag="wks")
        wvs = persist.tile([128, KD, DG], dt.float16, tag="wvs")
        wo_s = persist.tile([128, MD, D], dt.float16, tag="wo")

        # --- input DMAs, ordered for earliest first matmul (the cost model
        # serializes transfers on one DMA_ENGINES slot, so order matters):
        # the prelude computes kT c0, qT c0, v[h0-1] ms0-3, kT c1, v ms4-7,
        # kT c2 -- each group's data lands just before PE reaches it.
        nc.sync.dma_start(out=bq_s[:], in_=bq_d[:])
        nc.sync.dma_start(out=wks[:, :, 0:128], in_=wk_r[:, :, 0:128])
        for k2 in range(0, KD, 2):  # chunk 0 split so QK matmuls start early
            nc.sync.dma_start(out=xt_s[:, k2:k2 + 2, 0:CW[0]],
                              in_=xt_r[:, k2:k2 + 2, 0:CW[0]])
        nc.sync.dma_start(out=wqs[:, :, 0:128], in_=wq_r[:, :, 0:128])
        nc.sync.dma_start(out=wvs[:, :, 0:128], in_=wv_r[:, :, 0:128])
        nc.sync.dma_start(out=xt_s[:, :, CO[1]:CO[1] + CW[1]],
                          in_=xt_r[:, :, CO[1]:CO[1] + CW[1]])
        nc.sync.dma_start(out=xt_s[:, :, CO[2]:CO[2] + CW[2]],
                          in_=xt_r[:, :, CO[2]:CO[2] + CW[2]])
        for m in range(1, MD):
            sl = slice(m * 128, (m + 1) * 128)
            nc.sync.dma_start(out=wks[:, :, sl], in_=wk_r[:, :, sl])
            nc.sync.dma_start(out=wqs[:, :, sl], in_=wq_r[:, :, sl])
            nc.sync.dma_start(out=wvs[:, :, sl], in_=wv_r[:, :, sl])

        ones1 = persist.tile([128, 1], dt.float16, tag="ones1")
        nc.vector.memset(ones1[:], 1.0)
        nc.vector.tensor_copy(v[:, :, :, DH:DH + 1],
                              ones1[:].to_broadcast([128, KS, HPG, 1]))

        # ---- micro building blocks (one ps1 PSUM group each) -----------
        def qk_micro(m, which, n):
            """One sq chunk of the q or k projection for d-tile m (~2.1us)."""
            ws = wqs if which == "q" else wks
            cw, co = CW[n], CO[n]
            ps = ps1.tile([128, 1, 512], dt.float32, tag="ps1", name="ps1")
            for kk in range(KD):
                nc.tensor.matmul(
                    ps[:, 0, 0:cw],
                    lhsT=ws[:, kk, m * 128:(m + 1) * 128],
                    rhs=xt_s[:, kk, co:co + cw],
                    start=(kk == 0), stop=(kk == KD - 1))
            if which == "q":
                nc.vector.tensor_scalar(
                    qT[:, m, co:co + cw], ps[:, 0, 0:cw], 0.125,
                    bq_s[:, m:m + 1], op0=ALU.mult, op1=ALU.add)
            else:
                nc.vector.tensor_copy(kT[:, m, co:co + cw], ps[:, 0, 0:cw])

        def v_micro(hp, ms):
            """v columns for head pair hp, one 128-row s tile (~0.6us)."""
            sp = _sk(ms)
            ps = ps1.tile([128, 1, 512], dt.float32, tag="ps1", name="ps1")
            for kk in range(KD):
                nc.tensor.matmul(
                    ps[0:sp, 0, 0:128],
                    lhsT=xt_s[:, kk, ms * 128:ms * 128 + sp],
                    rhs=wvs[:, kk, hp * 128:(hp + 1) * 128],
                    start=(kk == 0), stop=(kk == KD - 1))
            nc.vector.tensor_copy(
                v[0:sp, ms, 2 * hp:2 * hp + 2, 0:DH],
                ps[0:sp, 0, 0:128].rearrange("p (h e) -> p h e", h=2))

        def wo_micro():
            nc.sync.dma_start(out=wo_s[:], in_=wo_r[:])

        def op_micro(ms, j):
            """One 512-col group of the O-projection for sq tile ms."""
            sp = _sk(ms)
            nw, noff = ON[j], OO[j]
            ps = ps1.tile([128, 1, 512], dt.float32, tag="ps1", name="ps1")
            for kk in range(MD):
                nc.tensor.matmul(
                    ps[0:sp, 0, 0:nw],
                    lhsT=ctxT[:, kk, ms * 128:ms * 128 + sp],
                    rhs=wo_s[:, kk, noff:noff + nw],
                    start=(kk == 0), stop=(kk == MD - 1))
            ob = opool.tile([128, 512], dt.float16, tag="ob", name="ob")
            nc.vector.tensor_copy(ob[0:sp, 0:nw], ps[0:sp, 0, 0:nw])
            # Mid-kernel out-stores ride Pool/SWDGE (SP.SEQ is busy with ctx
            # transposes whose sem waits hold it); the final sq tiles
            # alternate queues so the drain overlaps.
            eng = nc.gpsimd if (ms < 8 or j % 2 == 0) else nc.sync
            eng.dma_start(
                out=out_d[ms * 128:ms * 128 + sp, noff:noff + nw],
                in_=ob[0:sp, 0:nw])

        # ---- attention unit pieces -------------------------------------
        def emit_scores(h, c, pump):
            base = 64 * (h % 2)
            td = h // 2
            cw, co = CW[c], CO[c]
            csl = slice(co, co + cw)
            ex = epool.tile([128, KS, 512], dt.float16, tag="expT", name="ex")
            for kk2 in range(0, KS, 2):
                ps = ps2.tile([128, 2, 512], dt.float32, tag="ps2", name="ps2")
                for j in range(2):
                    kk = kk2 + j
                    sp = _sk(kk)
                    nc.tensor.matmul(
                        ps[0:sp, j, 0:cw],
                        lhsT=kT[base:base + 64, td, kk * 128:kk * 128 + sp],
                        rhs=qT[base:base + 64, td, csl],
                        start=True, stop=True)
                nc.scalar.activation(ex[:, kk2:kk2 + 2, 0:cw], ps[:, :, 0:cw],
                                     AF.Exp)
                pump()
            return ex

        csb_live = {}

        def emit_tail(h, c, ex):
            """attnV (ex stationary) + 1/Z scale into the pair's ctx_sb."""
            td, hb = h // 2, 64 * (h % 2)
            if (td, c) not in csb_live:
                csb_live[(td, c)] = {
                    off: cpool.tile([128, 128], dt.float16, tag="ctxsb",
                                    name="ctxsb")
                    for off, _ in _subtiles(c)}
            csb = csb_live[(td, c)]
            for off, sw in _subtiles(c):
                pc = pat.tile([128, DH + 1], dt.float32, tag="pat", name="pat")
                for kk in range(KS):
                    sp = _sk(kk)
                    nc.tensor.matmul(
                        pc[0:sw, :],
                        lhsT=ex[0:sp, kk, off:off + sw],
                        rhs=v[0:sp, kk, h, :],
                        start=(kk == 0), stop=(kk == KS - 1))
                rz = zpool.tile([128, 1], dt.float32, tag="rz", name="rz")
                nc.vector.reciprocal(rz[0:sw, :], pc[0:sw, DH:DH + 1])
                nc.vector.tensor_scalar(
                    csb[off][0:sw, hb:hb + 64], pc[0:sw, 0:DH], rz[0:sw, :],
                    None, op0=ALU.mult)
            if h % 2 == 1:  # pair complete: transpose ctx into ctxT
                for off, sw in _subtiles(c):
                    nc.sync.dma_start(
                        out=ctxT[:, td, CO[c] + off:CO[c] + off + sw],
                        in_=csb[off][0:sw, :], transpose=True)
                del csb_live[(td, c)]

        # ---- schedule --------------------------------------------------
        # c-major pair order: all c0 pairs first => O-proj for sq<512 can
        # start as filler at iteration 11, sq<1024 at 21.
        pairs = [(td, c) for c in (0, 1, 2) for td in range(5)]
        units = [(2 * td + o, c) for td, c in pairs for o in (0, 1)]

        # micro list: (cost_rows, dl, nb, fn); consumed strictly in order.
        # dl: must be emitted before scores of that iteration (RAW via
        # emission order).  nb: not before that iteration (transpose gating).
        M = []
        for ms in range(8, KS):
            M.append((1280, 1, 0, lambda ms=ms: v_micro(0, ms)))
        for m in range(1, MD):
            for n in range(NS):
                M.append((10 * CW[n], 2 * m, 0,
                          lambda m=m, n=n: qk_micro(m, "k", n)))
            M.append((10 * CW[0], 2 * m, 0, lambda m=m: qk_micro(m, "q", 0)))
            for ms in range(KS):
                M.append((1280, 2 * m + 1, 0,
                          lambda m=m, ms=ms: v_micro(m, ms)))
        M.append((0, 9, 0, wo_micro))
        for m in range(0, MD):
            M.append((10 * CW[1], 10 + 2 * m, max(0, 8 + 2 * m),
                      lambda m=m: qk_micro(m, "q", 1)))
            if m == 1:
                for ms in (0, 1):
                    for j in range(NS):
                        M.append((5 * ON[j], 29, 11,
                                  lambda ms=ms, j=j: op_micro(ms, j)))
            if m == 2:
                for ms in (2, 3):
                    for j in range(NS):
                        M.append((5 * ON[j], 29, 11,
                                  lambda ms=ms, j=j: op_micro(ms, j)))
        for m in range(0, MD):
            M.append((10 * CW[2], 20 + 2 * m, 18 + 2 * m,
                      lambda m=m: qk_micro(m, "q", 2)))
            if m in (1, 2):
                for ms in (2 * m + 2, 2 * m + 3):
                    for j in range(NS):
                        M.append((5 * ON[j], 29, 21,
                                  lambda ms=ms, j=j: op_micro(ms, j)))
        mq = deque(M)
        pace = sum(c for c, _, _, _ in M) / (len(units) * 6.0)

        state = {"iter": 0, "debt": 0.0}

        def drain_deadlines():
            # pop through the LAST due micro (due ones may sit behind
            # not-yet-due ops in the strictly-ordered queue)
            it = state["iter"]
            idx = -1
            for k, m in enumerate(mq):
                if m[1] <= it:
                    idx = k
            for _ in range(idx + 1):
                _, _, nb, fn = mq.popleft()
                assert nb <= it, "nb violation forced by a deadline"
                fn()

        def pump():
            state["debt"] += pace
            while mq and state["debt"] > 0 and mq[0][2] <= state["iter"]:
                cost, _, _, fn = mq.popleft()
                fn()
                state["debt"] -= cost

        # prelude: kT d-tile 0 (scores need all of kT) + qT d-tile 0 chunk 0,
        # with v[h0-1] micros filling the waits for the xt c1/c2 DMAs
        qk_micro(0, "k", 0)
        qk_micro(0, "q", 0)
        for ms in range(0, 4):
            v_micro(0, ms)
        qk_micro(0, "k", 1)
        for ms in range(4, 8):
            v_micro(0, ms)
        qk_micro(0, "k", 2)

        exm = {}
        for i, u in enumerate(units):
            state["iter"] = i
            drain_deadlines()
            exm[u] = emit_scores(u[0], u[1], pump)
            if i >= 1:
                up = units[i - 1]
                emit_tail(up[0], up[1], exm.pop(up))
        up = units[-1]
        emit_tail(up[0], up[1], exm.pop(up))
        state["iter"] = len(units)
        while mq:
            _, _, _, fn = mq.popleft()
            fn()
        for ms in range(8, KS):
            for j in range(NS):
                op_micro(ms, j)

    nc.compile()
    return nc


def _get_nc():
    if "nc" not in _CACHE:
        _CACHE["nc"] = build()
    return _CACHE["nc"]


def _prep_in_maps(x, Wq, bq, Wk, Wv, Wo):
    in_maps = []
    for c in range(N_CORES):
        b, g = divmod(c, G)
        gs = slice(g * DG, (g + 1) * DG)
        in_maps.append({
            "xt": np.ascontiguousarray(x[b].T).astype(np.float16),
            "wq": np.ascontiguousarray(Wq[gs, :].T).astype(np.float16),
            "wk": np.ascontiguousarray(Wk[gs, :].T).astype(np.float16),
            "wv": np.ascontiguousarray(Wv[gs, :].T).astype(np.float16),
            "wo": np.ascontiguousarray(Wo[:, gs].T).astype(np.float16),
            "bq": np.ascontiguousarray(
                (0.125 * bq[gs]).astype(np.float32).reshape(MD, 128).T),
        })
    return in_maps


def run(x, Wq, bq, Wk, Wv, bv, Wo, bo, trace=False, **trace_kw):
    x = np.asarray(x, dtype=np.float32)
    Wq = np.asarray(Wq, dtype=np.float32)
    bq = np.asarray(bq, dtype=np.float32)
    Wk = np.asarray(Wk, dtype=np.float32)
    Wv = np.asarray(Wv, dtype=np.float32)
    bv = np.asarray(bv, dtype=np.float32)
    Wo = np.asarray(Wo, dtype=np.float32)
    bo = np.asarray(bo, dtype=np.float32)

    nc = _get_nc()
    in_maps = _prep_in_maps(x, Wq, bq, Wk, Wv, Wo)
    res = None
    for attempt in range(3):
        try:
            res = run_bass_kernel_spmd(nc, in_maps, list(range(N_CORES)),
                                       trace=trace, **trace_kw)
            break
        except Exception:
            # Sporadic NRT_EXEC_UNIT_UNRECOVERABLE on first exec; devices
            # come back after ~75s. Reset the backend and retry.
            if attempt == 2:
                raise
            import time as _time
            import jax as _jax
            _time.sleep(80)
            try:
                _jax.clear_backends()
            except Exception:
                pass
    const = (bv @ Wo.T + bo).astype(np.float32)  # [D]
    out = np.empty((B, S, D), dtype=np.float32)
    for b in range(B):
        out[b] = (res.results[2 * b]["out"].astype(np.float32)
                  + res.results[2 * b + 1]["out"].astype(np.float32) + const)
    return out, res


def kernel(**inputs):
    out, _ = run(**inputs)
    return out
